# revision 11
# baseline (speedup 1.0000x reference)
"""Trainium2 Bass kernel for 16-head MHA (B=4, N=2048, E=1024), 8-core SPMD.

Sharding: core c owns batch b = c//2 and 8 of 16 heads (half = c%2).
Per core, everything is computed with zero on-device transposes:
  - host passes x[b].T (bf16), head-gathered Wq/Wk/Wv (Wq pre-scaled by
    1/sqrt(E)), and the head-sharded rows of Wproj.
  - qT/kT are produced in [d, tokens] layout directly (W as stationary),
    v in [tokens, d] layout (xT as stationary).
  - scores are computed transposed, S[k, q] = kT_tile^T @ qT (stationary kT),
    so softmax-exp output feeds AV as the moving operand with contraction on
    the partition axis, and AV emits att_out^T[d, q] (stationary v).
  - softmax skips max-subtraction (scores ~ N(0, 1/16), exp is safe in fp32);
    the normalizer Z is folded into AV via a ones-column on v (M=65).
  - output projection consumes att^T pair tiles [128(2 heads*64d), q] and
    emits outT[e_out, tokens] partials; the host sums the 2-core pairs and
    transposes back.
"""

import numpy as np
import ml_dtypes

B, N, E, H, D = 4, 2048, 1024, 16, 64
HPC = 8      # heads per core
NPAIR = 4    # head-pairs per core
NET = 8      # contraction e-tiles of 128
NTT = 16     # token tiles of 128
NQC = 4      # q-chunks of 512
NKT = 16     # k tiles of 128
bf16 = ml_dtypes.bfloat16

_CACHE = {}
LAST = {"exec_time_ns": None, "mean_exec_time_ns": None, "profile": None}


def _split_multiwaits(nc, mybir):
    """This walrus build accepts only one sync-wait per instruction; hoist
    extra waits onto single-wait NOPs inserted just before the instruction."""
    n_new = 0
    for f in nc.m.functions:
        for bb in f.blocks:
            insts = bb.instructions
            i = 0
            while i < len(insts):
                ins = insts[i]
                si = getattr(ins, "sync_info", None)
                if si is not None and si.on_wait is not None and len(si.on_wait) > 1:
                    waits = list(si.on_wait)
                    keep, extra = waits[-1], waits[:-1]
                    new_nops = []
                    for w in extra:
                        n_new += 1
                        nop = mybir.InstNoOp(
                            name=f"{ins.name}-wsplit{n_new}",
                            sync_info=mybir.SyncInfo(on_wait=[w], on_update=[]),
                            engine=ins.engine,
                            bass_nofuse=True,
                        )
                        nc.register_instruction(nop, overwrite=True)
                        new_nops.append(nop)
                    si.on_wait.clear()
                    si.on_wait.append(keep)
                    insts[i:i] = new_nops
                    i += len(new_nops)
                i += 1
    return n_new


def _build_model():
    import concourse.bass as bass
    import concourse.mybir as mybir
    import concourse.tile as tile

    F32, BF16 = mybir.dt.float32, mybir.dt.bfloat16
    Exp = mybir.ActivationFunctionType.Exp
    ts = bass.ts

    nc = bass.Bass()
    xT_dr = nc.dram_tensor("xT", [E, N], BF16, kind="ExternalInput")
    wq_dr = nc.dram_tensor("wq", [E, HPC * D], BF16, kind="ExternalInput")
    wk_dr = nc.dram_tensor("wk", [E, HPC * D], BF16, kind="ExternalInput")
    wv_dr = nc.dram_tensor("wv", [E, HPC * D], BF16, kind="ExternalInput")
    wp_dr = nc.dram_tensor("wp", [HPC * D, E], BF16, kind="ExternalInput")
    bqk_dr = nc.dram_tensor("bqk", [128, 2 * NPAIR], F32, kind="ExternalInput")
    bv_dr = nc.dram_tensor("bv", [1, HPC * D], BF16, kind="ExternalInput")
    bp_dr = nc.dram_tensor("bp", [128, NET], F32, kind="ExternalInput")
    o_dr = nc.dram_tensor("o", [E, N], mybir.dt.float16, kind="ExternalOutput")

    with tile.TileContext(nc) as tc:
        with (
            tc.tile_pool(name="const", bufs=1) as const,
            tc.tile_pool(name="exp_sb", bufs=14) as exp_pool,
            tc.tile_pool(name="small", bufs=4) as small,
            tc.tile_pool(name="evict", bufs=4) as evict,
            tc.tile_pool(name="dram", bufs=4, space="DRAM") as drp,
            tc.tile_pool(name="ps_s", bufs=2, space="PSUM") as ps_s,
            tc.tile_pool(name="ps_acc", bufs=2, space="PSUM") as ps_acc,
            tc.tile_pool(name="ps_mm", bufs=2, space="PSUM") as ps_mm,
        ):
            # ---- constant loads ----
            # Issue order matters: the first q-projection group needs wq[et]
            # and xT[et], so interleave those first to minimize the prologue.
            xT_sb = const.tile([128, NET, N], BF16)
            wq_sb = const.tile([128, NET, HPC * D], BF16)
            wk_sb = const.tile([128, NET, HPC * D], BF16)
            wv_sb = const.tile([128, NET, HPC * D], BF16)
            # HWDGE descriptor processing is one serialized pipe (~0.6us/DMA),
            # so SP carries ONLY the first-window critical path (wk0, wq0, all
            # of xT) and everything else rides the separate SWDGE (gpsimd)
            # pipe, biases first (they gate the qT/kT evictions).
            xr = xT_dr.rearrange("(a p) n -> p a n", p=128)
            wqr = wq_dr.rearrange("(a p) n -> p a n", p=128)
            wkr = wk_dr.rearrange("(a p) n -> p a n", p=128)
            nc.sync.dma_start(wk_sb[:, 0, :], wkr[:, 0, :])
            nc.sync.dma_start(wq_sb[:, 0, :], wqr[:, 0, :])
            for et in range(NET):
                # Alternate queues (SP / ACT HWDGE) so the 512KB transfers
                # overlap instead of serializing on one queue.
                eng = nc.sync if et % 2 == 0 else nc.scalar
                eng.dma_start(xT_sb[:, et, :], xr[:, et, :])
            bqk_sb = const.tile([128, 2 * NPAIR], F32)
            nc.gpsimd.dma_start(bqk_sb[:], bqk_dr[:])
            bv_sb = const.tile([1, HPC * D], BF16)
            nc.gpsimd.dma_start(bv_sb[:], bv_dr[:])
            wvr = wv_dr.rearrange("(a p) n -> p a n", p=128)
            for et in range(1, NET):
                nc.gpsimd.dma_start(wk_sb[:, et, :], wkr[:, et, :])
                nc.gpsimd.dma_start(wq_sb[:, et, :], wqr[:, et, :])
            for et in range(NET):
                nc.gpsimd.dma_start(wv_sb[:, et, :], wvr[:, et, :])
            wp_sb = const.tile([128, NPAIR, E], BF16)
            wpr = wp_dr.rearrange("(a p) n -> p a n", p=128)
            for g in range(NPAIR):
                nc.gpsimd.dma_start(wp_sb[:, g, :], wpr[:, g, :])
            bp_sb = const.tile([128, NET], F32)
            nc.gpsimd.dma_start(bp_sb[:], bp_dr[:])
            ones_sb = const.tile([1, 128], BF16)
            nc.vector.memset(ones_sb[:], 1.0)

            qT_sb = const.tile([128, NPAIR, N], BF16)
            kT_sb = const.tile([128, NPAIR, N], BF16)
            FP8 = mybir.dt.float8e4
            q8_sb = const.tile([128, NPAIR, 2, N], FP8)
            k8_sb = const.tile([128, NPAIR, N], FP8)
            Copy = mybir.ActivationFunctionType.Copy
            # v tiles: [token-tile, head, 65]; d at 0:64, ones column at 64
            # (the ones column folds the softmax normalizer Z into AV).
            v_sb = const.tile([128, NTT, HPC, D + 1], BF16)
            att_sb = const.tile([128, NPAIR, N], BF16)

            for h in range(HPC):
                nc.vector.memset(v_sb[:, :, h, D], 1.0)

            # PE warm-up: the HAM clock gate starts at 1.2GHz and needs ~3.4us
            # of sustained activity to release to 2.4GHz. Burn that time on
            # dummy matmuls over a memset tile while the xT/wq/wk DMAs land,
            # so the first real groups run at full clock.
            warm_sb = const.tile([128, 512], BF16)
            nc.vector.memset(warm_sb[:], 0.0)
            wps = ps_mm.tile([128, 512], F32, tag="mm", name="warm_ps")
            for i in range(10):
                nc.tensor.matmul(
                    wps[:], warm_sb[:, 0:128], warm_sb[:], start=(i == 0), stop=(i == 9)
                )
            # ACT warm-up: a dummy exp hoists the ~2.7us activation-table load
            # off the first real exp's critical path into the DMA wait.
            warm_e = const.tile([128, 1], BF16)
            nc.scalar.activation(warm_e[:], warm_sb[:, 0:1], Exp)

            def qk_group(p, which, qc):
                w_sb = wk_sb if which == "k" else wq_sb
                bias_col = NPAIR + p if which == "k" else p
                dst = kT_sb if which == "k" else qT_sb
                ps = ps_mm.tile([128, 512], F32, tag="mm")
                for et in range(NET):
                    nc.tensor.matmul(
                        ps[:],
                        w_sb[:, et, ts(p, 128)],
                        xT_sb[:, et, ts(qc, 512)],
                        start=(et == 0),
                        stop=(et == NET - 1),
                    )
                nc.vector.tensor_scalar_add(
                    dst[:, p, ts(qc, 512)], ps[:], bqk_sb[:, bias_col : bias_col + 1]
                )
                if which == "k":
                    nc.scalar.activation(
                        k8_sb[:, p, ts(qc, 512)], dst[:, p, ts(qc, 512)], Copy
                    )
                else:
                    nc.scalar.activation(
                        q8_sb[:, p, 0, ts(qc, 512)], dst[:, p, ts(qc, 512)], Copy
                    )
                    nc.vector.tensor_tensor(
                        q8_sb[:, p, 1, ts(qc, 512)],
                        dst[:, p, ts(qc, 512)],
                        q8_sb[:, p, 0, ts(qc, 512)],
                        mybir.AluOpType.subtract,
                    )

            def v_tile(p, tt, npair=1):
                # Computes v for `npair` consecutive pairs starting at p in one
                # matmul group (wider moving operand = fewer instructions).
                np_ = npair * 128
                ps = ps_mm.tile([128, np_], F32, tag="mm")
                for et in range(NET):
                    nc.tensor.matmul(
                        ps[:],
                        xT_sb[:, et, ts(tt, 128)],
                        wv_sb[:, et, p * 128 : p * 128 + np_],
                        start=(et == 0),
                        stop=False,
                    )
                nc.tensor.matmul(
                    ps[:],
                    ones_sb[0:1, 0:128],
                    bv_sb[0:1, p * 128 : p * 128 + np_],
                    start=False,
                    stop=True,
                )
                for g in range(npair):
                    nc.vector.tensor_copy(
                        v_sb[:, tt, 2 * (p + g), 0:64], ps[:, g * 128 : g * 128 + 64]
                    )
                    nc.vector.tensor_copy(
                        v_sb[:, tt, 2 * (p + g) + 1, 0:64], ps[:, g * 128 + 64 : g * 128 + 128]
                    )

            def qkv_pair(p):
                # Emission order tuned so the first attention window of the
                # pair unblocks as early as possible: scores(p, qc0, kt) only
                # needs kT chunk kt//4 and qT(p, qc0), so emit k-chunk 0 and
                # q-chunk 0 first; later k chunks and v tiles are consumed
                # kt-ascending a few microseconds later. v for pairs 1-3 is
                # computed in one wide pass during pair 1's slot.
                qk_group(p, "k", 0)
                qk_group(p, "q", 0)
                for qc in range(1, NQC):
                    qk_group(p, "k", qc)
                for tt in range(NTT):
                    v_tile(p, tt, npair=1)
                for qc in range(1, NQC):
                    qk_group(p, "q", qc)


            PIPE_LAG = 5  # trailing-work items the AV/divide stream lags by

            def make_window(p, qc):
                """Closures for one (pair, q-chunk) window: a scores/exp
                emitter, an AV emitter (lazy accumulator allocation), and the
                softmax-divide eviction."""
                h0, h1 = 2 * p, 2 * p + 1
                state = {}

                def scores(ktp):
                    kt0, kt1 = 2 * ktp, 2 * ktp + 1
                    sA = ps_s.tile([128, 2, 512], F32, tag="s")
                    sB = ps_s.tile([128, 2, 512], F32, tag="s")
                    for i, kt in enumerate((kt0, kt1)):
                        for base_p, sbuf in ((0, sA), (64, sB)):
                            kr = k8_sb[base_p : base_p + 64, p, ts(kt, 128)]
                            qr = q8_sb[base_p : base_p + 64, p, 0, ts(qc, 512)]
                            nc.tensor.matmul(
                                sbuf[:, i, :],
                                bass.AP(tensor=kr.tensor, offset=kr.offset,
                                        ap=[kr.ap[0], [0, 2]] + kr.ap[1:]),
                                bass.AP(tensor=qr.tensor, offset=qr.offset,
                                        ap=[qr.ap[0], [N, 2]] + qr.ap[1:]),
                                start=True,
                                stop=True,
                                perf_mode=mybir.MatmulPerfMode.DoubleRow,
                            )
                    eA = exp_pool.tile([128, 2, 512], BF16, tag="e")
                    eB = exp_pool.tile([128, 2, 512], BF16, tag="e")
                    nc.scalar.activation(eA[:], sA[:], Exp, scale=float(1.0 / 32.0))
                    nc.scalar.activation(eB[:], sB[:], Exp, scale=float(1.0 / 32.0))
                    state[ktp] = (eA, eB)

                def av(ktp):
                    if "avA" not in state:
                        state["avA"] = ps_acc.tile([65, 512], F32, tag="acc", name=f"avA_{p}_{qc}")
                        state["avB"] = ps_acc.tile([65, 512], F32, tag="acc", name=f"avB_{p}_{qc}")
                    avA, avB = state["avA"], state["avB"]
                    kt0, kt1 = 2 * ktp, 2 * ktp + 1
                    eA, eB = state.pop(ktp)
                    for i, kt in enumerate((kt0, kt1)):
                        nc.tensor.matmul(
                            avA[:],
                            v_sb[:, kt, h0, :],
                            eA[:, i, :],
                            start=(kt == 0),
                            stop=(kt == NKT - 1),
                        )
                    for i, kt in enumerate((kt0, kt1)):
                        nc.tensor.matmul(
                            avB[:],
                            v_sb[:, kt, h1, :],
                            eB[:, i, :],
                            start=(kt == 0),
                            stop=(kt == NKT - 1),
                        )

                def divide():
                    avA, avB = state.pop("avA"), state.pop("avB")
                    # h0: av rows 0:64 = out*d, row 64 = Z
                    rzA = small.tile([128, 512], F32, tag="rz")
                    nc.vector.reciprocal(rzA[64:65, :], avA[64:65, :])
                    rdA = drp.tile([1, 512], F32, tag="rzd")
                    nc.sync.dma_start(rdA[:], rzA[64:65, :])
                    bcA = bass.AP(
                        tensor=rdA.tensor,
                        offset=rdA[0:1, :].offset,
                        ap=[[0, 64]] + rdA[0:1, :].ap[1:],
                    )
                    nc.sync.dma_start(rzA[0:64, :], bcA)
                    nc.vector.tensor_mul(
                        att_sb[0:64, p, ts(qc, 512)], avA[0:64, :], rzA[0:64, :]
                    )
                    # h1: same layout, then partition-shift DMA into rows
                    # 64:128 of the att pair tile.
                    rzB = small.tile([128, 512], F32, tag="rz")
                    nc.vector.reciprocal(rzB[64:65, :], avB[64:65, :])
                    rdB = drp.tile([1, 512], F32, tag="rzd")
                    nc.sync.dma_start(rdB[:], rzB[64:65, :])
                    bcB = bass.AP(
                        tensor=rdB.tensor,
                        offset=rdB[0:1, :].offset,
                        ap=[[0, 64]] + rdB[0:1, :].ap[1:],
                    )
                    nc.sync.dma_start(rzB[0:64, :], bcB)
                    tmp = small.tile([64, 512], BF16, tag="atmp")
                    nc.vector.tensor_mul(tmp[:], avB[0:64, :], rzB[0:64, :])
                    nc.sync.dma_start(att_sb[64:128, p, ts(qc, 512)], tmp[:])

                return scores, av, divide

            # ---- output projection: outT[eout, t] = sum_pairs wp^T @ attT ----
            orr = o_dr.rearrange("(a p) n -> p a n", p=128)

            def proj_qc(tcn):
                for eo in range(NET):
                    ps = ps_mm.tile([128, 512], F32, tag="mm")
                    for g in range(NPAIR):
                        nc.tensor.matmul(
                            ps[:],
                            wp_sb[:, g, ts(eo, 128)],
                            att_sb[:, g, ts(tcn, 512)],
                            start=(g == 0),
                            stop=(g == NPAIR - 1),
                        )
                    ot = evict.tile([128, 512], mybir.dt.float16, tag="o")
                    nc.vector.tensor_scalar_add(ot[:], ps[:], bp_sb[:, eo : eo + 1])
                    nc.sync.dma_start(orr[:, eo, ts(tcn, 512)], ot[:])

            # ---- schedule: qkv(p) then attention(p); qkv(p+1) fills PE gaps;
            # proj for q-chunk tcn starts as soon as the LAST pair's window for
            # tcn completes (all other pairs' att for tcn is long done) ----
            # proj_qc(qc) is emitted AFTER window(3, qc+1) so the next window's
            # score fills outrank the projection burst in PE priority.
            # Software-pipelined emission: the scores/exp stream runs
            # continuously across window boundaries while AV matmuls and the
            # softmax divide trail through a small FIFO, so the in-order PE
            # stream never puts blocking tail-work ahead of the next window's
            # score fills. qkv of the next pair and the projection chunks are
            # spliced between windows as PE gap-filler.
            fifo = []

            def drain_to(depth):
                while len(fifo) > depth:
                    fifo.pop(0)()

            qkv_pair(0)
            for p in range(NPAIR):
                nx = p + 1
                for qc in range(NQC):
                    scores, av, divide = make_window(p, qc)
                    for ktp in range(NKT // 2):
                        scores(ktp)
                        fifo.append(lambda ktp=ktp, av=av: av(ktp))
                        drain_to(PIPE_LAG)
                    fifo.append(divide)
                    if nx < NPAIR:
                        if qc == 1:
                            qk_group(nx, "k", 0)
                            qk_group(nx, "q", 0)
                        elif qc == 2:
                            for c in range(1, NQC):
                                qk_group(nx, "k", c)
                        elif qc == 3:
                            for tt in range(NTT):
                                v_tile(nx, tt)
                            for c in range(1, NQC):
                                qk_group(nx, "q", c)
                    if p == NPAIR - 1 and qc >= 1:
                        drain_to(0)
                        proj_qc(qc - 1)
            drain_to(0)
            proj_qc(NQC - 1)

    _split_multiwaits(nc, mybir)
    return nc


def _host_prep(x, Wqkv, bqkv, Wproj, bproj):
    x = np.asarray(x, dtype=np.float32)
    Wqkv = np.asarray(Wqkv, dtype=np.float32)
    bqkv = np.asarray(bqkv, dtype=np.float32)
    Wproj = np.asarray(Wproj, dtype=np.float32)
    bproj = np.asarray(bproj, dtype=np.float32)

    scale = 1.0 / np.sqrt(np.float32(E))
    cols = np.arange(E)
    hh, dd = cols // D, cols % D
    qcol = hh * (3 * D) + dd * 3 + 0
    kcol = hh * (3 * D) + dd * 3 + 1
    vcol = hh * (3 * D) + dd * 3 + 2
    Wq = Wqkv[:, qcol].astype(bf16)
    Wk = Wqkv[:, kcol].astype(bf16)
    Wv = Wqkv[:, vcol].astype(bf16)
    bq = bqkv[qcol]
    bk = bqkv[kcol]
    bv = bqkv[vcol].astype(bf16)

    in_maps = []
    for c in range(8):
        b, half = c // 2, c % 2
        sl = slice(half * HPC * D, (half + 1) * HPC * D)
        bqk = np.zeros((128, 2 * NPAIR), np.float32)
        bqs, bks = bq[sl], bk[sl]
        for p in range(NPAIR):
            bqk[:, p] = bqs[p * 128 : (p + 1) * 128]
            bqk[:, NPAIR + p] = bks[p * 128 : (p + 1) * 128]
        bp = np.zeros((128, NET), np.float32)
        if half == 0:
            bp[:] = bproj.reshape(NET, 128).T
        in_maps.append(
            {
                "xT": np.ascontiguousarray(x[b].T).astype(bf16),
                "wq": np.ascontiguousarray(Wq[:, sl]),
                "wk": np.ascontiguousarray(Wk[:, sl]),
                "wv": np.ascontiguousarray(Wv[:, sl]),
                "wp": np.ascontiguousarray(Wproj[sl, :]).astype(bf16),
                "bqk": bqk,
                "bv": np.ascontiguousarray(bv[sl]).reshape(1, HPC * D),
                "bp": bp,
            }
        )
    return in_maps


def kernel(x, Wqkv, bqkv, Wproj, bproj):
    import os
    from concourse.bass_utils import run_bass_kernel_spmd

    if "nc" not in _CACHE:
        _CACHE["nc"] = _build_model()
    nc = _CACHE["nc"]

    in_maps = _host_prep(x, Wqkv, bqkv, Wproj, bproj)
    trace = bool(int(os.environ.get("TRN_TRACE", "0")))
    if trace:
        try:
            res = run_bass_kernel_spmd(nc, in_maps, core_ids=list(range(8)), trace=True)
        except Exception:
            trace = False
    if not trace:
        # Retry transient device failures (NRT_EXEC_UNIT_UNRECOVERABLE and
        # sporadic all-NaN outputs have been observed; both clear on rerun).
        last_exc = None
        for attempt in range(4):
            try:
                res = run_bass_kernel_spmd(nc, in_maps, core_ids=list(range(8)))
                bad = any(
                    not np.isfinite(np.asarray(r["o"], dtype=np.float32)).all()
                    for r in res.results
                )
                if not bad:
                    break
                last_exc = RuntimeError("non-finite device output")
            except Exception as e:
                last_exc = e
            import time as _time

            _time.sleep(2.0 * (attempt + 1))
        else:
            raise last_exc
    LAST["exec_time_ns"] = res.exec_time_ns
    LAST["mean_exec_time_ns"] = res.mean_exec_time_ns
    LAST["profile"] = res.profile_json

    out = np.empty((B, N, E), np.float32)
    for b in range(B):
        oT = res.results[2 * b]["o"].astype(np.float32) + res.results[2 * b + 1]["o"].astype(np.float32)
        out[b] = oT.T
    return out



# revision 12
# speedup vs baseline: 1.0468x; 1.0468x over previous
"""Trainium2 Bass kernel for 16-head MHA (B=4, N=2048, E=1024), 8-core SPMD.

Sharding: core c owns batch b = c//2 and 8 of 16 heads (half = c%2).
Per core, everything is computed with zero on-device transposes:
  - host passes x[b].T (bf16), head-gathered Wq/Wk/Wv (Wq pre-scaled by
    1/sqrt(E)), and the head-sharded rows of Wproj.
  - qT/kT are produced in [d, tokens] layout directly (W as stationary),
    v in [tokens, d] layout (xT as stationary).
  - scores are computed transposed, S[k, q] = kT_tile^T @ qT (stationary kT),
    so softmax-exp output feeds AV as the moving operand with contraction on
    the partition axis, and AV emits att_out^T[d, q] (stationary v).
  - softmax skips max-subtraction (scores ~ N(0, 1/16), exp is safe in fp32);
    the normalizer Z is folded into AV via a ones-column on v (M=65).
  - output projection consumes att^T pair tiles [128(2 heads*64d), q] and
    emits outT[e_out, tokens] partials; the host sums the 2-core pairs and
    transposes back.
"""

import numpy as np
import ml_dtypes

B, N, E, H, D = 4, 2048, 1024, 16, 64
HPC = 8      # heads per core
NPAIR = 4    # head-pairs per core
NET = 8      # contraction e-tiles of 128
NTT = 16     # token tiles of 128
NQC = 4      # q-chunks of 512
NKT = 16     # k tiles of 128
bf16 = ml_dtypes.bfloat16

_CACHE = {}
LAST = {"exec_time_ns": None, "mean_exec_time_ns": None, "profile": None}


def _split_multiwaits(nc, mybir):
    """This walrus build accepts only one sync-wait per instruction; hoist
    extra waits onto single-wait NOPs inserted just before the instruction."""
    n_new = 0
    for f in nc.m.functions:
        for bb in f.blocks:
            insts = bb.instructions
            i = 0
            while i < len(insts):
                ins = insts[i]
                si = getattr(ins, "sync_info", None)
                if si is not None and si.on_wait is not None and len(si.on_wait) > 1:
                    waits = list(si.on_wait)
                    keep, extra = waits[-1], waits[:-1]
                    new_nops = []
                    for w in extra:
                        n_new += 1
                        nop = mybir.InstNoOp(
                            name=f"{ins.name}-wsplit{n_new}",
                            sync_info=mybir.SyncInfo(on_wait=[w], on_update=[]),
                            engine=ins.engine,
                            bass_nofuse=True,
                        )
                        nc.register_instruction(nop, overwrite=True)
                        new_nops.append(nop)
                    si.on_wait.clear()
                    si.on_wait.append(keep)
                    insts[i:i] = new_nops
                    i += len(new_nops)
                i += 1
    return n_new


def _build_model():
    import concourse.bass as bass
    import concourse.mybir as mybir
    import concourse.tile as tile

    F32, BF16 = mybir.dt.float32, mybir.dt.bfloat16
    Exp = mybir.ActivationFunctionType.Exp
    ts = bass.ts

    nc = bass.Bass()
    xT_dr = nc.dram_tensor("xT", [E, N], BF16, kind="ExternalInput")
    wq_dr = nc.dram_tensor("wq", [E, HPC * D], BF16, kind="ExternalInput")
    wk_dr = nc.dram_tensor("wk", [E, HPC * D], BF16, kind="ExternalInput")
    wv_dr = nc.dram_tensor("wv", [E, HPC * D], BF16, kind="ExternalInput")
    wp_dr = nc.dram_tensor("wp", [HPC * D, E], BF16, kind="ExternalInput")
    bqk_dr = nc.dram_tensor("bqk", [128, 2 * NPAIR], F32, kind="ExternalInput")
    bv_dr = nc.dram_tensor("bv", [1, HPC * D], BF16, kind="ExternalInput")
    bp_dr = nc.dram_tensor("bp", [128, NET], F32, kind="ExternalInput")
    o_dr = nc.dram_tensor("o", [E, N], mybir.dt.float16, kind="ExternalOutput")

    with tile.TileContext(nc) as tc:
        with (
            tc.tile_pool(name="const", bufs=1) as const,
            tc.tile_pool(name="exp_sb", bufs=14) as exp_pool,
            tc.tile_pool(name="small", bufs=4) as small,
            tc.tile_pool(name="evict", bufs=4) as evict,
            tc.tile_pool(name="dram", bufs=4, space="DRAM") as drp,
            tc.tile_pool(name="ps_s", bufs=2, space="PSUM") as ps_s,
            tc.tile_pool(name="ps_acc", bufs=2, space="PSUM") as ps_acc,
            tc.tile_pool(name="ps_mm", bufs=2, space="PSUM") as ps_mm,
        ):
            # ---- constant loads ----
            # Issue order matters: the first q-projection group needs wq[et]
            # and xT[et], so interleave those first to minimize the prologue.
            xT_sb = const.tile([128, NET, N], BF16)
            wq_sb = const.tile([128, NET, HPC * D], BF16)
            wk_sb = const.tile([128, NET, HPC * D], BF16)
            wv_sb = const.tile([128, NET, HPC * D], BF16)
            # HWDGE descriptor processing is one serialized pipe (~0.6us/DMA),
            # so SP carries ONLY the first-window critical path (wk0, wq0, all
            # of xT) and everything else rides the separate SWDGE (gpsimd)
            # pipe, biases first (they gate the qT/kT evictions).
            xr = xT_dr.rearrange("(a p) n -> p a n", p=128)
            wqr = wq_dr.rearrange("(a p) n -> p a n", p=128)
            wkr = wk_dr.rearrange("(a p) n -> p a n", p=128)
            nc.sync.dma_start(wk_sb[:, 0, :], wkr[:, 0, :])
            nc.sync.dma_start(wq_sb[:, 0, :], wqr[:, 0, :])
            for et in range(NET):
                # Alternate queues (SP / ACT HWDGE) so the 512KB transfers
                # overlap instead of serializing on one queue.
                eng = nc.sync if et % 2 == 0 else nc.scalar
                eng.dma_start(xT_sb[:, et, :], xr[:, et, :])
            bqk_sb = const.tile([128, 2 * NPAIR], F32)
            nc.gpsimd.dma_start(bqk_sb[:], bqk_dr[:])
            bv_sb = const.tile([1, HPC * D], BF16)
            nc.gpsimd.dma_start(bv_sb[:], bv_dr[:])
            wvr = wv_dr.rearrange("(a p) n -> p a n", p=128)
            for et in range(1, NET):
                nc.gpsimd.dma_start(wk_sb[:, et, :], wkr[:, et, :])
                nc.gpsimd.dma_start(wq_sb[:, et, :], wqr[:, et, :])
            for et in range(NET):
                nc.gpsimd.dma_start(wv_sb[:, et, :], wvr[:, et, :])
            wp_sb = const.tile([128, NPAIR, E], BF16)
            wpr = wp_dr.rearrange("(a p) n -> p a n", p=128)
            for g in range(NPAIR):
                nc.gpsimd.dma_start(wp_sb[:, g, :], wpr[:, g, :])
            bp_sb = const.tile([128, NET], F32)
            nc.gpsimd.dma_start(bp_sb[:], bp_dr[:])
            ones_sb = const.tile([1, 128], BF16)
            nc.vector.memset(ones_sb[:], 1.0)

            qT_sb = const.tile([128, NPAIR, N], BF16)
            kT_sb = const.tile([128, NPAIR, N], BF16)
            FP8 = mybir.dt.float8e4
            q8_sb = const.tile([128, NPAIR, 2, N], FP8)
            k8_sb = const.tile([128, NPAIR, N], FP8)
            Copy = mybir.ActivationFunctionType.Copy
            # v tiles: [token-tile, head, 65]; d at 0:64, ones column at 64
            # (the ones column folds the softmax normalizer Z into AV).
            v_sb = const.tile([128, NTT, HPC, D + 1], BF16)
            att_sb = const.tile([128, NPAIR, N], BF16)

            for h in range(HPC):
                nc.vector.memset(v_sb[:, :, h, D], 1.0)

            # PE warm-up: the HAM clock gate starts at 1.2GHz and needs ~3.4us
            # of sustained activity to release to 2.4GHz. Burn that time on
            # dummy matmuls over a memset tile while the xT/wq/wk DMAs land,
            # so the first real groups run at full clock.
            warm_sb = const.tile([128, 512], BF16)
            nc.vector.memset(warm_sb[:], 0.0)
            wps = ps_mm.tile([128, 512], F32, tag="mm", name="warm_ps")
            for i in range(10):
                nc.tensor.matmul(
                    wps[:], warm_sb[:, 0:128], warm_sb[:], start=(i == 0), stop=(i == 9)
                )
            # ACT warm-up: a dummy exp hoists the ~2.7us activation-table load
            # off the first real exp's critical path into the DMA wait.
            warm_e = const.tile([128, 1], BF16)
            nc.scalar.activation(warm_e[:], warm_sb[:, 0:1], Exp)

            def qk_group(p, which, qc):
                w_sb = wk_sb if which == "k" else wq_sb
                bias_col = NPAIR + p if which == "k" else p
                dst = kT_sb if which == "k" else qT_sb
                ps = ps_mm.tile([128, 512], F32, tag="mm")
                for et in range(NET):
                    nc.tensor.matmul(
                        ps[:],
                        w_sb[:, et, ts(p, 128)],
                        xT_sb[:, et, ts(qc, 512)],
                        start=(et == 0),
                        stop=(et == NET - 1),
                    )
                nc.vector.tensor_scalar_add(
                    dst[:, p, ts(qc, 512)], ps[:], bqk_sb[:, bias_col : bias_col + 1]
                )
                if which == "k":
                    nc.vector.tensor_copy(
                        k8_sb[:, p, ts(qc, 512)], dst[:, p, ts(qc, 512)]
                    )
                else:
                    nc.vector.tensor_copy(
                        q8_sb[:, p, 0, ts(qc, 512)], dst[:, p, ts(qc, 512)]
                    )
                    nc.vector.tensor_tensor(
                        q8_sb[:, p, 1, ts(qc, 512)],
                        dst[:, p, ts(qc, 512)],
                        q8_sb[:, p, 0, ts(qc, 512)],
                        mybir.AluOpType.subtract,
                    )

            def v_tile(p, tt, npair=1):
                # Computes v for `npair` consecutive pairs starting at p in one
                # matmul group (wider moving operand = fewer instructions).
                np_ = npair * 128
                ps = ps_mm.tile([128, np_], F32, tag="mm")
                for et in range(NET):
                    nc.tensor.matmul(
                        ps[:],
                        xT_sb[:, et, ts(tt, 128)],
                        wv_sb[:, et, p * 128 : p * 128 + np_],
                        start=(et == 0),
                        stop=False,
                    )
                nc.tensor.matmul(
                    ps[:],
                    ones_sb[0:1, 0:128],
                    bv_sb[0:1, p * 128 : p * 128 + np_],
                    start=False,
                    stop=True,
                )
                for g in range(npair):
                    nc.vector.tensor_copy(
                        v_sb[:, tt, 2 * (p + g), 0:64], ps[:, g * 128 : g * 128 + 64]
                    )
                    nc.vector.tensor_copy(
                        v_sb[:, tt, 2 * (p + g) + 1, 0:64], ps[:, g * 128 + 64 : g * 128 + 128]
                    )

            def qkv_pair(p):
                # Emission order tuned so the first attention window of the
                # pair unblocks as early as possible: scores(p, qc0, kt) only
                # needs kT chunk kt//4 and qT(p, qc0), so emit k-chunk 0 and
                # q-chunk 0 first; later k chunks and v tiles are consumed
                # kt-ascending a few microseconds later. v for pairs 1-3 is
                # computed in one wide pass during pair 1's slot.
                qk_group(p, "k", 0)
                qk_group(p, "q", 0)
                for qc in range(1, NQC):
                    qk_group(p, "k", qc)
                for tt in range(NTT):
                    v_tile(p, tt, npair=1)
                for qc in range(1, NQC):
                    qk_group(p, "q", qc)


            PIPE_LAG = 5  # trailing-work items the AV/divide stream lags by

            def make_window(p, qc):
                """Closures for one (pair, q-chunk) window: a scores/exp
                emitter, an AV emitter (lazy accumulator allocation), and the
                softmax-divide eviction."""
                h0, h1 = 2 * p, 2 * p + 1
                state = {}

                def scores(ktp):
                    kt0, kt1 = 2 * ktp, 2 * ktp + 1
                    sA = ps_s.tile([128, 2, 512], F32, tag="s")
                    sB = ps_s.tile([128, 2, 512], F32, tag="s")
                    for i, kt in enumerate((kt0, kt1)):
                        for base_p, sbuf in ((0, sA), (64, sB)):
                            kr = k8_sb[base_p : base_p + 64, p, ts(kt, 128)]
                            qr = q8_sb[base_p : base_p + 64, p, 0, ts(qc, 512)]
                            nc.tensor.matmul(
                                sbuf[:, i, :],
                                bass.AP(tensor=kr.tensor, offset=kr.offset,
                                        ap=[kr.ap[0], [0, 2]] + kr.ap[1:]),
                                bass.AP(tensor=qr.tensor, offset=qr.offset,
                                        ap=[qr.ap[0], [N, 2]] + qr.ap[1:]),
                                start=True,
                                stop=True,
                                perf_mode=mybir.MatmulPerfMode.DoubleRow,
                            )
                    eA = exp_pool.tile([128, 2, 512], BF16, tag="e")
                    eB = exp_pool.tile([128, 2, 512], BF16, tag="e")
                    nc.scalar.activation(eA[:], sA[:], Exp, scale=float(1.0 / 32.0))
                    nc.scalar.activation(eB[:], sB[:], Exp, scale=float(1.0 / 32.0))
                    state[ktp] = (eA, eB)

                def av(ktp):
                    if "avA" not in state:
                        state["avA"] = ps_acc.tile([65, 512], F32, tag="acc", name=f"avA_{p}_{qc}")
                        state["avB"] = ps_acc.tile([65, 512], F32, tag="acc", name=f"avB_{p}_{qc}")
                    avA, avB = state["avA"], state["avB"]
                    kt0, kt1 = 2 * ktp, 2 * ktp + 1
                    eA, eB = state.pop(ktp)
                    for i, kt in enumerate((kt0, kt1)):
                        nc.tensor.matmul(
                            avA[:],
                            v_sb[:, kt, h0, :],
                            eA[:, i, :],
                            start=(kt == 0),
                            stop=(kt == NKT - 1),
                        )
                    for i, kt in enumerate((kt0, kt1)):
                        nc.tensor.matmul(
                            avB[:],
                            v_sb[:, kt, h1, :],
                            eB[:, i, :],
                            start=(kt == 0),
                            stop=(kt == NKT - 1),
                        )

                def divide():
                    avA, avB = state.pop("avA"), state.pop("avB")
                    # h0: av rows 0:64 = out*d, row 64 = Z
                    rzA = small.tile([128, 512], F32, tag="rz")
                    nc.vector.reciprocal(rzA[64:65, :], avA[64:65, :])
                    rdA = drp.tile([1, 512], F32, tag="rzd")
                    nc.sync.dma_start(rdA[:], rzA[64:65, :])
                    bcA = bass.AP(
                        tensor=rdA.tensor,
                        offset=rdA[0:1, :].offset,
                        ap=[[0, 64]] + rdA[0:1, :].ap[1:],
                    )
                    nc.sync.dma_start(rzA[0:64, :], bcA)
                    nc.vector.tensor_mul(
                        att_sb[0:64, p, ts(qc, 512)], avA[0:64, :], rzA[0:64, :]
                    )
                    # h1: same layout, then partition-shift DMA into rows
                    # 64:128 of the att pair tile.
                    rzB = small.tile([128, 512], F32, tag="rz")
                    nc.vector.reciprocal(rzB[64:65, :], avB[64:65, :])
                    rdB = drp.tile([1, 512], F32, tag="rzd")
                    nc.sync.dma_start(rdB[:], rzB[64:65, :])
                    bcB = bass.AP(
                        tensor=rdB.tensor,
                        offset=rdB[0:1, :].offset,
                        ap=[[0, 64]] + rdB[0:1, :].ap[1:],
                    )
                    nc.sync.dma_start(rzB[0:64, :], bcB)
                    tmp = small.tile([64, 512], BF16, tag="atmp")
                    nc.vector.tensor_mul(tmp[:], avB[0:64, :], rzB[0:64, :])
                    nc.sync.dma_start(att_sb[64:128, p, ts(qc, 512)], tmp[:])

                return scores, av, divide

            # ---- output projection: outT[eout, t] = sum_pairs wp^T @ attT ----
            orr = o_dr.rearrange("(a p) n -> p a n", p=128)

            def proj_qc(tcn):
                for eo in range(NET):
                    ps = ps_mm.tile([128, 512], F32, tag="mm")
                    for g in range(NPAIR):
                        nc.tensor.matmul(
                            ps[:],
                            wp_sb[:, g, ts(eo, 128)],
                            att_sb[:, g, ts(tcn, 512)],
                            start=(g == 0),
                            stop=(g == NPAIR - 1),
                        )
                    ot = evict.tile([128, 512], mybir.dt.float16, tag="o")
                    nc.vector.tensor_scalar_add(ot[:], ps[:], bp_sb[:, eo : eo + 1])
                    nc.sync.dma_start(orr[:, eo, ts(tcn, 512)], ot[:])

            # ---- schedule: qkv(p) then attention(p); qkv(p+1) fills PE gaps;
            # proj for q-chunk tcn starts as soon as the LAST pair's window for
            # tcn completes (all other pairs' att for tcn is long done) ----
            # proj_qc(qc) is emitted AFTER window(3, qc+1) so the next window's
            # score fills outrank the projection burst in PE priority.
            # Software-pipelined emission: the scores/exp stream runs
            # continuously across window boundaries while AV matmuls and the
            # softmax divide trail through a small FIFO, so the in-order PE
            # stream never puts blocking tail-work ahead of the next window's
            # score fills. qkv of the next pair and the projection chunks are
            # spliced between windows as PE gap-filler.
            fifo = []

            def drain_to(depth):
                while len(fifo) > depth:
                    fifo.pop(0)()

            qkv_pair(0)
            for p in range(NPAIR):
                nx = p + 1
                for qc in range(NQC):
                    scores, av, divide = make_window(p, qc)
                    for ktp in range(NKT // 2):
                        scores(ktp)
                        fifo.append(lambda ktp=ktp, av=av: av(ktp))
                        drain_to(PIPE_LAG)
                    fifo.append(divide)
                    if nx < NPAIR:
                        if qc == 1:
                            qk_group(nx, "k", 0)
                            qk_group(nx, "q", 0)
                        elif qc == 2:
                            for c in range(1, NQC):
                                qk_group(nx, "k", c)
                        elif qc == 3:
                            for tt in range(NTT):
                                v_tile(nx, tt)
                            for c in range(1, NQC):
                                qk_group(nx, "q", c)
                    if p == NPAIR - 1 and qc >= 1:
                        drain_to(0)
                        proj_qc(qc - 1)
            drain_to(0)
            proj_qc(NQC - 1)

    _split_multiwaits(nc, mybir)
    return nc


def _host_prep(x, Wqkv, bqkv, Wproj, bproj):
    x = np.asarray(x, dtype=np.float32)
    Wqkv = np.asarray(Wqkv, dtype=np.float32)
    bqkv = np.asarray(bqkv, dtype=np.float32)
    Wproj = np.asarray(Wproj, dtype=np.float32)
    bproj = np.asarray(bproj, dtype=np.float32)

    scale = 1.0 / np.sqrt(np.float32(E))
    cols = np.arange(E)
    hh, dd = cols // D, cols % D
    qcol = hh * (3 * D) + dd * 3 + 0
    kcol = hh * (3 * D) + dd * 3 + 1
    vcol = hh * (3 * D) + dd * 3 + 2
    Wq = Wqkv[:, qcol].astype(bf16)
    Wk = Wqkv[:, kcol].astype(bf16)
    Wv = Wqkv[:, vcol].astype(bf16)
    bq = bqkv[qcol]
    bk = bqkv[kcol]
    bv = bqkv[vcol].astype(bf16)

    in_maps = []
    for c in range(8):
        b, half = c // 2, c % 2
        sl = slice(half * HPC * D, (half + 1) * HPC * D)
        bqk = np.zeros((128, 2 * NPAIR), np.float32)
        bqs, bks = bq[sl], bk[sl]
        for p in range(NPAIR):
            bqk[:, p] = bqs[p * 128 : (p + 1) * 128]
            bqk[:, NPAIR + p] = bks[p * 128 : (p + 1) * 128]
        bp = np.zeros((128, NET), np.float32)
        if half == 0:
            bp[:] = bproj.reshape(NET, 128).T
        in_maps.append(
            {
                "xT": np.ascontiguousarray(x[b].T).astype(bf16),
                "wq": np.ascontiguousarray(Wq[:, sl]),
                "wk": np.ascontiguousarray(Wk[:, sl]),
                "wv": np.ascontiguousarray(Wv[:, sl]),
                "wp": np.ascontiguousarray(Wproj[sl, :]).astype(bf16),
                "bqk": bqk,
                "bv": np.ascontiguousarray(bv[sl]).reshape(1, HPC * D),
                "bp": bp,
            }
        )
    return in_maps


def kernel(x, Wqkv, bqkv, Wproj, bproj):
    import os
    from concourse.bass_utils import run_bass_kernel_spmd

    if "nc" not in _CACHE:
        _CACHE["nc"] = _build_model()
    nc = _CACHE["nc"]

    in_maps = _host_prep(x, Wqkv, bqkv, Wproj, bproj)
    trace = bool(int(os.environ.get("TRN_TRACE", "0")))
    if trace:
        try:
            res = run_bass_kernel_spmd(nc, in_maps, core_ids=list(range(8)), trace=True)
        except Exception:
            trace = False
    if not trace:
        # Retry transient device failures (NRT_EXEC_UNIT_UNRECOVERABLE and
        # sporadic all-NaN outputs have been observed; both clear on rerun).
        last_exc = None
        for attempt in range(4):
            try:
                res = run_bass_kernel_spmd(nc, in_maps, core_ids=list(range(8)))
                bad = any(
                    not np.isfinite(np.asarray(r["o"], dtype=np.float32)).all()
                    for r in res.results
                )
                if not bad:
                    break
                last_exc = RuntimeError("non-finite device output")
            except Exception as e:
                last_exc = e
            import time as _time

            _time.sleep(2.0 * (attempt + 1))
        else:
            raise last_exc
    LAST["exec_time_ns"] = res.exec_time_ns
    LAST["mean_exec_time_ns"] = res.mean_exec_time_ns
    LAST["profile"] = res.profile_json

    out = np.empty((B, N, E), np.float32)
    for b in range(B):
        oT = res.results[2 * b]["o"].astype(np.float32) + res.results[2 * b + 1]["o"].astype(np.float32)
        out[b] = oT.T
    return out

